# revision 9
# baseline (speedup 1.0000x reference)
"""Trainium2 Bass kernel for nn_BoundaryGreenBranch.

Strategy (8 NeuronCores, full inputs in / full output out):
  - Shard the 64x64 coarse grid by rows: core k owns a 10-row window
    (640 coarse points, 2 rows of overlap so each core can run its own
    slice of the bilinear upsample -> zero cross-core communication) and
    produces output rows [32k, 32k+32) of the final [4,1,256,256].
  - Per core, all 512 (batch, boundary-point) pairs are processed with two
    boundary points stacked on the 128 partitions (2 x 64 hidden).  The
    green-kernel MLP runs entirely out of SBUF/PSUM (flash-style, nothing
    materialized in HBM):
      mm1   K=4  [cx; cy; d0; d1] x W4            -> h1_pre  [128, 640]
      gelu1 (+ per-pair bias a = bf@g1w_f + g1b, per-partition bias)
      mm2   K=128 blockdiag(g2w, g2w)             -> h2_pre  [64, 640]
      gelu2 (+ blockdiag bias)
      mm3   K=128 blockdiag4(g3w)                 -> raw     [8, 640] / 4 pairs
    Distances for all pairs are precomputed with one rank-3 matmul per batch
    plus Sqrt/Exp activations.  The weighted sum over boundary points is a
    single K=128 PE reduction per batch at the end, followed by the separable
    bilinear upsample done as two small matmuls per batch.
"""

import numpy as np
import ml_dtypes

import concourse.bass as bass
import concourse.mybir as mybir
import concourse.tile as tile
from concourse import bacc
from concourse.bass_utils import run_bass_kernel_spmd

B, NBC, HID = 4, 128, 64
H = W = 256
HC = WC = 64
CF = 4
NCORES = 8
RPC = 10                 # coarse rows per core (incl. upsample overlap)
MK = RPC * WC            # 640 coarse points per core
OUT_ROWS = H // NCORES   # 32 output rows per core
NPAIR = B * NBC // 2     # 256 pairs of boundary points
EPS = 1e-8

F32 = mybir.dt.float32
BF16 = mybir.dt.bfloat16
AF = mybir.ActivationFunctionType
ALU = mybir.AluOpType

LAST_RESULT = None       # BassKernelResults of the most recent run (for test.py)
TRACE = False            # set True by test.py to capture an NTFF profile


def _core_row_starts():
    starts = []
    for k in range(NCORES):
        s = (OUT_ROWS * k * (HC - 1)) // (H - 1)
        starts.append(min(s, HC - RPC))
    return starts


def _interp_matrix(out_idx, n_in, lo, n_win, n_out_total):
    out_idx = list(out_idx)
    R = np.zeros((len(out_idx), n_win), dtype=np.float64)
    for i, h in enumerate(out_idx):
        y = h * (n_in - 1) / (n_out_total - 1)
        y0 = int(np.floor(y))
        y1 = min(y0 + 1, n_in - 1)
        fy = y - y0
        assert lo <= y0 and y1 < lo + n_win
        R[i, y0 - lo] += 1.0 - fy
        R[i, y1 - lo] += fy
    return R


def _build_program():
    nc = bacc.Bacc("TRN2")

    def din(name, shape, dtype=F32):
        return nc.dram_tensor(name, list(shape), dtype, kind="ExternalInput")

    d_binfo = din("binfo", [B, NBC, 3])
    d_binfoT = din("binfoT", [3, B * NBC])
    d_lpre = din("lpre", [3, B * NBC])  # rows [bx, by, -0.5]; L3 = -2 * lpre
    d_e1w = din("e1w", [3, HID])
    d_e1b = din("e1b", [HID, 1])
    d_e2w = din("e2w", [HID, HID])
    d_e2b = din("e2b", [HID, 1])
    d_g1wf = din("g1wf", [HID, HID])
    d_g1b = din("g1b", [HID, 1])
    d_w4 = din("w4", [4, 128], BF16)
    d_g2bd = din("g2bd", [128, HID], BF16)
    d_g2b2 = din("g2b2", [128, 1])
    d_g3a = din("g3a", [128, 8], BF16)
    d_g3b_ = din("g3bm", [128, 8], BF16)
    d_g3b4 = din("g3b4", [4, 1])
    d_eye4 = din("eye4", [128, 16])
    d_cxd3 = din("cxd3", [3, MK])
    d_xcy = din("xcy", [2, MK], BF16)
    d_ryt = din("ryt", [RPC, OUT_ROWS])
    d_rx = din("rx", [HC, W])
    d_ds = din("ds", [1, 1])
    d_out = nc.dram_tensor("out", [B, OUT_ROWS, W], F32, kind="ExternalOutput")

    CH = [(0, 512), (512, 640)]  # PSUM-bank-sized free-dim chunks of MK

    with tile.TileContext(nc) as tc:
        with (
            tc.tile_pool(name="const", bufs=1) as cp,
            tc.tile_pool(name="persist", bufs=1) as pp,
        ):
            def cload(dram, shape, dtype=F32, name=None):
                t = cp.tile(shape, dtype, name=name or dram.name + "_sb")
                nc.sync.dma_start(out=t, in_=dram[:])
                return t

            sb_binfoT = cload(d_binfoT, [3, B * NBC])
            sb_lpre = cload(d_lpre, [3, B * NBC])
            sb_e1w = cload(d_e1w, [3, HID])
            sb_e1b = cload(d_e1b, [HID, 1])
            sb_e2w = cload(d_e2w, [HID, HID])
            sb_e2b = cload(d_e2b, [HID, 1])
            sb_g1wf = cload(d_g1wf, [HID, HID])
            sb_g1b = cload(d_g1b, [HID, 1])
            sb_w4 = cload(d_w4, [4, 128], BF16)
            sb_g2bd = cload(d_g2bd, [128, HID], BF16)
            sb_g2b2 = cload(d_g2b2, [128, 1])
            sb_g3a = cload(d_g3a, [128, 8], BF16)
            sb_g3b_ = cload(d_g3b_, [128, 8], BF16)
            sb_g3b4 = cload(d_g3b4, [4, 1])
            sb_eye4 = cload(d_eye4, [128, 16])
            sb_cxd3 = cload(d_cxd3, [3, MK])
            sb_ryt = cload(d_ryt, [RPC, OUT_ROWS])
            sb_rx = cload(d_rx, [HC, W])
            sb_binfo = cp.tile([NBC, B * 3], F32, name="binfo_sb")
            for b in range(B):
                nc.sync.dma_start(out=sb_binfo[:, 3 * b:3 * b + 3], in_=d_binfo[b])
            sb_s = cp.tile([128, 1], F32, name="s_sb")
            nc.sync.dma_start(
                out=sb_s,
                in_=bass.AP(tensor=d_ds, offset=0, ap=[[0, 128], [1, 1]]),
            )

            # persistent intermediates
            DW = [pp.tile([NBC, MK], F32, name=f"dw{b}") for b in range(B)]
            DBF = [pp.tile([NBC, MK], BF16, name=f"dbf{b}") for b in range(B)]
            RAW = [pp.tile([NBC, MK], F32, name=f"raw{b}") for b in range(B)]
            A_col = pp.tile([128, NPAIR], F32, name="a_col")

            # ---------------- preamble: distances, then encoder ----------
            with (
                tc.tile_pool(name="pre_sb", bufs=2) as sp,
                tc.tile_pool(name="pre_ps", bufs=2, space="PSUM") as pq,
            ):
                # -|s| on all partitions
                s_abs = sp.tile([128, 1], F32, name="s_abs")
                nc.scalar.activation(s_abs, sb_s, AF.Abs)
                s_neg = sp.tile([128, 1], F32, name="s_neg")
                nc.vector.tensor_scalar_mul(s_neg, s_abs, -1.0)

                # L3 rows: [-2bx; -2by; ones]  over all 512 boundary points
                L3 = sp.tile([3, B * NBC], F32, name="L3")
                nc.vector.tensor_scalar_mul(L3, sb_lpre, -2.0)

                # per-partition bias bx^2 + by^2 + eps  (column per batch)
                bxy = sp.tile([NBC, B], F32, name="bxy")
                for b in range(B):
                    sq = sp.tile([NBC, 2], F32, name="sq")
                    nc.vector.tensor_mul(
                        sq, sb_binfo[:, 3 * b:3 * b + 2], sb_binfo[:, 3 * b:3 * b + 2]
                    )
                    nc.vector.tensor_reduce(
                        bxy[:, b:b + 1], sq, axis=mybir.AxisListType.X, op=ALU.add
                    )
                nc.vector.tensor_scalar_add(bxy, bxy, EPS)

                # dist2 -> dist -> dw (+bf16 cast of dist)
                dist32 = []
                ps_d = []
                for b in range(B):
                    ps = pq.tile([NBC, MK], F32, name="pps", tag="pps")
                    for lo, hi in CH:
                        nc.tensor.matmul(
                            ps[:, lo:hi],
                            lhsT=L3[:, NBC * b:NBC * (b + 1)],
                            rhs=sb_cxd3[:, lo:hi],
                            start=True,
                            stop=True,
                        )
                    ps_d.append(ps)
                for b in range(B):
                    dst = sp.tile([NBC, MK], F32, name=f"dist32_{b}", tag=f"d32_{b}")
                    nc.scalar.activation(
                        dst, ps_d[b], AF.Sqrt, bias=bxy[:, b:b + 1]
                    )
                    dist32.append(dst)
                for b in range(B):
                    nc.scalar.activation(
                        DW[b], dist32[b], AF.Exp, scale=s_neg[:, 0:1]
                    )
                for b in range(B):
                    nc.vector.tensor_copy(DBF[b], dist32[b])

                # boundary encoder (fp32): bf = gelu(gelu(x@e1+b)@e2+b)
                ps1 = pq.tile([HID, B * NBC], F32, name="pps_e1", tag="pps")
                nc.tensor.matmul(ps1, lhsT=sb_e1w, rhs=sb_binfoT, start=True, stop=True)
                enc1 = sp.tile([HID, B * NBC], F32, name="enc1")
                nc.scalar.activation(enc1, ps1, AF.Gelu, bias=sb_e1b[:, 0:1])
                ps2 = pq.tile([HID, B * NBC], F32, name="pps_e2", tag="pps")
                nc.tensor.matmul(ps2, lhsT=sb_e2w, rhs=enc1, start=True, stop=True)
                bf = sp.tile([HID, B * NBC], F32, name="bf")
                nc.scalar.activation(bf, ps2, AF.Gelu, bias=sb_e2b[:, 0:1])
                ps3 = pq.tile([HID, B * NBC], F32, name="pps_a", tag="pps")
                nc.tensor.matmul(ps3, lhsT=sb_g1wf, rhs=bf, start=True, stop=True)
                A = sp.tile([HID, B * NBC], F32, name="A")
                nc.scalar.activation(A, ps3, AF.Identity, bias=sb_g1b[:, 0:1])

                # A_col [128, 256]: column p = concat(a[:, 2p], a[:, 2p+1])
                Av = A.rearrange("h (p two) -> h two p", two=2)
                nc.sync.dma_start(out=A_col[0:HID, :], in_=Av[:, 0, :])
                nc.sync.dma_start(out=A_col[HID:128, :], in_=Av[:, 1, :])

            # ---------------- main loop ----------------------------------
            with (
                tc.tile_pool(name="xi", bufs=2) as xip,
                tc.tile_pool(name="h1p", bufs=2) as h1p,
                tc.tile_pool(name="h2wp", bufs=2) as h2wp,
                tc.tile_pool(name="stgp", bufs=4) as stgp,
                tc.tile_pool(name="ps_h1", bufs=1, space="PSUM") as psh1,
                tc.tile_pool(name="ps_h2", bufs=2, space="PSUM") as psh2,
                tc.tile_pool(name="ps_raw", bufs=1, space="PSUM") as psraw,
            ):
                for g in range(8):
                    b, half = g // 2, g % 2
                    xi = xip.tile([4, 32 * MK], BF16, name="xi", tag="xi")
                    xiv = xi.rearrange("r (q m) -> r q m", m=MK)
                    nc.sync.dma_start(
                        out=xiv[0:2],
                        in_=bass.AP(
                            tensor=d_xcy, offset=0, ap=[[MK, 2], [0, 32], [1, MK]]
                        ),
                    )
                    dv = DBF[b][64 * half:64 * half + 64, :].rearrange(
                        "(q r) m -> q r m", r=2
                    )
                    nc.sync.dma_start(out=xiv[2:3], in_=dv[:, 0, :])
                    nc.sync.dma_start(out=xiv[3:4], in_=dv[:, 1, :])

                    ph2 = None
                    praw = None
                    for q in range(32):
                        pair = 32 * g + q
                        ph1 = psh1.tile([128, MK], F32, name="ph1", tag="ph1")
                        for lo, hi in CH:
                            nc.tensor.matmul(
                                ph1[:, lo:hi],
                                lhsT=sb_w4,
                                rhs=xi[:, MK * q + lo:MK * q + hi],
                                start=True,
                                stop=True,
                            )
                        h1 = h1p.tile([128, MK], BF16, name="h1", tag="h1")
                        nc.scalar.activation(
                            h1, ph1, AF.Gelu, bias=A_col[:, pair:pair + 1]
                        )
                        if q % 2 == 0:
                            ph2 = psh2.tile([128, MK], F32, name="ph2", tag="ph2")
                        p0 = 64 * (q % 2)
                        for lo, hi in CH:
                            nc.tensor.matmul(
                                ph2[p0:p0 + 64, lo:hi],
                                lhsT=sb_g2bd,
                                rhs=h1[:, lo:hi],
                                start=True,
                                stop=True,
                            )
                        if q % 2 == 1:
                            h2w = h2wp.tile([128, MK], BF16, name="h2w", tag="h2w")
                            nc.scalar.activation(
                                h2w, ph2, AF.Gelu, bias=sb_g2b2[:, 0:1]
                            )
                            if q % 4 == 1:
                                praw = psraw.tile([8, MK], F32, name="praw", tag="praw")
                            wsel = sb_g3a if q % 4 == 1 else sb_g3b_
                            for lo, hi in CH:
                                nc.tensor.matmul(
                                    praw[:, lo:hi],
                                    lhsT=wsel,
                                    rhs=h2w[:, lo:hi],
                                    start=(q % 4 == 1),
                                    stop=(q % 4 == 3),
                                    skip_group_check=True,
                                )
                            if q % 4 == 3:
                                stg = stgp.tile([8, MK], F32, name="stg", tag="stg")
                                nc.vector.tensor_copy(stg, praw)
                                r0 = 64 * half + 2 * (q - 3)
                                nc.sync.dma_start(
                                    out=RAW[b][r0:r0 + 8, :], in_=stg
                                )

            # ---------------- epilogue -----------------------------------
            with (
                tc.tile_pool(name="epi_sb", bufs=2) as ep,
                tc.tile_pool(name="epi_ps", bufs=1, space="PSUM") as eq,
            ):
                for b in range(B):
                    nc.vector.tensor_mul(RAW[b], RAW[b], DW[b])
                ps_u1 = eq.tile([B, MK], F32, name="ps_u1", tag="u1")
                ps_u2 = eq.tile([B, MK], F32, name="ps_u2", tag="u2")
                for b in range(B):
                    for lo, hi in CH:
                        nc.tensor.matmul(
                            ps_u1[:, lo:hi],
                            lhsT=sb_eye4[:, 4 * b:4 * b + 4],
                            rhs=RAW[b][:, lo:hi],
                            start=(b == 0),
                            stop=(b == B - 1),
                            skip_group_check=True,
                        )
                for b in range(B):
                    for lo, hi in CH:
                        nc.tensor.matmul(
                            ps_u2[:, lo:hi],
                            lhsT=sb_eye4[:, 4 * b:4 * b + 4],
                            rhs=DW[b][:, lo:hi],
                            start=(b == 0),
                            stop=(b == B - 1),
                            skip_group_check=True,
                        )
                u_sb = ep.tile([B, MK], F32, name="u_sb")
                nc.vector.tensor_scalar(
                    u_sb, ps_u2, sb_g3b4[:, 0:1], None, op0=ALU.mult
                )
                nc.vector.tensor_add(u_sb, u_sb, ps_u1)

                for b in range(B):
                    ub = ep.tile([RPC, WC], F32, name=f"ub{b}", tag="ub")
                    nc.sync.dma_start(out=ub, in_=u_sb[b:b + 1, :])
                    ps_c = eq.tile([WC, OUT_ROWS], F32, name="ps_c", tag="psc")
                    nc.tensor.matmul(ps_c, lhsT=ub, rhs=sb_ryt, start=True, stop=True)
                    c1t = ep.tile([WC, OUT_ROWS], F32, name="c1t", tag="c1t")
                    nc.vector.tensor_copy(c1t, ps_c)
                    ps_o = eq.tile([OUT_ROWS, W], F32, name="ps_o", tag="pso")
                    nc.tensor.matmul(ps_o, lhsT=c1t, rhs=sb_rx, start=True, stop=True)
                    o_sb = ep.tile([OUT_ROWS, W], F32, name=f"o{b}", tag="osb")
                    nc.vector.tensor_copy(o_sb, ps_o)
                    nc.sync.dma_start(out=d_out[b], in_=o_sb)

    nc.finalize()
    return nc


_CACHED = None


def _get_program():
    global _CACHED
    if _CACHED is None:
        _CACHED = _build_program()
    return _CACHED


def _make_in_maps(inputs):
    f32 = lambda x: np.ascontiguousarray(np.asarray(x), dtype=np.float32)
    b16 = lambda x: np.ascontiguousarray(
        np.asarray(x, dtype=np.float32).astype(ml_dtypes.bfloat16)
    )
    binfo = f32(inputs["boundary_info"])
    e1w, e1b = f32(inputs["e1w"]), f32(inputs["e1b"])
    e2w, e2b = f32(inputs["e2w"]), f32(inputs["e2b"])
    g1w, g1b = f32(inputs["g1w"]), f32(inputs["g1b"])
    g2w, g2b = f32(inputs["g2w"]), f32(inputs["g2b"])
    g3w, g3b = f32(inputs["g3w"]), f32(inputs["g3b"])
    ds = f32(inputs["distance_scale"])

    gxw, gyw, gdw = g1w[HID + 0], g1w[HID + 1], g1w[HID + 2]
    w4 = np.zeros((4, 128), np.float32)
    w4[0, :HID], w4[0, HID:] = gxw, gxw
    w4[1, :HID], w4[1, HID:] = gyw, gyw
    w4[2, :HID] = gdw
    w4[3, HID:] = gdw

    g2bd = np.zeros((128, HID), np.float32)
    g2bd[:HID, :32] = g2w
    g2bd[HID:, 32:] = g2w
    g2b2 = np.tile(g2b, 4)[:, None]

    g3a = np.zeros((128, 8), np.float32)
    g3bm = np.zeros((128, 8), np.float32)
    for j in range(4):
        g3a[32 * j:32 * j + 32, j] = g3w[:, 0]
        g3bm[32 * j:32 * j + 32, 4 + j] = g3w[:, 0]

    eye4 = np.zeros((128, 16), np.float32)
    for b in range(4):
        eye4[:, 4 * b + b] = 1.0

    gx = np.linspace(-1.0, 1.0, WC)
    gy = np.linspace(-1.0, 1.0, HC)
    rx = np.ascontiguousarray(
        _interp_matrix(range(W), WC, 0, WC, W).T.astype(np.float32)
    )  # [64, 256]

    binfoT = np.ascontiguousarray(binfo.reshape(B * NBC, 3).T)
    lpre = binfoT.copy()
    lpre[2, :] = -0.5
    shared = dict(
        binfo=binfo,
        binfoT=binfoT,
        lpre=lpre,
        e1w=e1w,
        e1b=np.ascontiguousarray(e1b[:, None]),
        e2w=e2w,
        e2b=np.ascontiguousarray(e2b[:, None]),
        g1wf=np.ascontiguousarray(g1w[:HID]),
        g1b=np.ascontiguousarray(g1b[:, None]),
        w4=b16(w4),
        g2bd=b16(g2bd),
        g2b2=f32(g2b2),
        g3a=b16(g3a),
        g3bm=b16(g3bm),
        g3b4=np.full((4, 1), g3b[0], np.float32),
        eye4=eye4,
        rx=rx,
        ds=ds.reshape(1, 1),
    )

    starts = _core_row_starts()
    in_maps = []
    for k in range(NCORES):
        sk = starts[k]
        rows = np.arange(sk, sk + RPC)
        cy = np.repeat(gy[rows], WC)
        cx = np.tile(gx, RPC)
        cxd3 = np.stack([cx, cy, cx * cx + cy * cy]).astype(np.float32)
        xcy = np.stack([cx, cy]).astype(np.float32)
        ryt = (
            _interp_matrix(range(OUT_ROWS * k, OUT_ROWS * (k + 1)), HC, sk, RPC, H)
            / NBC
        ).T.astype(np.float32)  # [10, 32]
        m = dict(shared)
        m.update(
            cxd3=np.ascontiguousarray(cxd3),
            xcy=b16(xcy),
            ryt=np.ascontiguousarray(ryt),
        )
        in_maps.append(m)
    return in_maps


def kernel(**inputs) -> np.ndarray:
    global LAST_RESULT
    assert int(inputs["H"]) == H and int(inputs["W"]) == W
    nc = _get_program()
    in_maps = _make_in_maps(inputs)
    res = run_bass_kernel_spmd(
        nc, in_maps, core_ids=list(range(NCORES)), trace=TRACE
    )
    LAST_RESULT = res
    shards = [r["out"] for r in res.results]  # each [B, 32, W]
    out = np.concatenate(shards, axis=1)      # [B, 256, W]
    return out[:, None, :, :].astype(np.float32)


# revision 14
# speedup vs baseline: 1.4005x; 1.4005x over previous
"""Trainium2 Bass kernel for nn_BoundaryGreenBranch.

Strategy (8 NeuronCores, full inputs in / full output out):
  - Shard the 64x64 coarse grid by rows: core k owns a 10-row window
    (640 coarse points, 2 rows of overlap so each core can run its own
    slice of the bilinear upsample -> zero cross-core communication) and
    produces output rows [32k, 32k+32) of the final [4,1,256,256].
  - Per core, all 512 (batch, boundary-point) pairs are processed with two
    boundary points stacked on the 128 partitions (2 x 64 hidden).  The
    green-kernel MLP runs entirely out of SBUF/PSUM (flash-style, nothing
    materialized in HBM):
      mm1   K=4  [cx; cy; d0; d1] x W4            -> h1_pre  [128, 640]
      gelu1 (+ per-pair bias a = bf@g1w_f + g1b, per-partition bias)
      mm2   K=128 blockdiag(g2w, g2w)             -> h2_pre  [64, 640]
      gelu2 (+ blockdiag bias)
      mm3   K=128 blockdiag4(g3w)                 -> raw     [8, 640] / 4 pairs
    Distances for all pairs are precomputed with one rank-3 matmul per batch
    plus Sqrt/Exp activations.  The weighted sum over boundary points is a
    single K=128 PE reduction per batch at the end, followed by the separable
    bilinear upsample done as two small matmuls per batch.
"""

import numpy as np
import ml_dtypes

import concourse.bass as bass
import concourse.mybir as mybir
import concourse.tile as tile
from concourse import bacc
from concourse.bass_utils import run_bass_kernel_spmd

B, NBC, HID = 4, 128, 64
H = W = 256
HC = WC = 64
CF = 4
NCORES = 8
RPC = 10                 # coarse rows per core (incl. upsample overlap)
MK = RPC * WC            # 640 coarse points per core
OUT_ROWS = H // NCORES   # 32 output rows per core
NPAIR = B * NBC // 2     # 256 pairs of boundary points
EPS = 1e-8

F32 = mybir.dt.float32
BF16 = mybir.dt.bfloat16
AF = mybir.ActivationFunctionType
ALU = mybir.AluOpType

LAST_RESULT = None       # BassKernelResults of the most recent run (for test.py)
TRACE = False            # set True by test.py to capture an NTFF profile


def _core_row_starts():
    starts = []
    for k in range(NCORES):
        s = (OUT_ROWS * k * (HC - 1)) // (H - 1)
        starts.append(min(s, HC - RPC))
    return starts


def _interp_matrix(out_idx, n_in, lo, n_win, n_out_total):
    out_idx = list(out_idx)
    R = np.zeros((len(out_idx), n_win), dtype=np.float64)
    for i, h in enumerate(out_idx):
        y = h * (n_in - 1) / (n_out_total - 1)
        y0 = int(np.floor(y))
        y1 = min(y0 + 1, n_in - 1)
        fy = y - y0
        assert lo <= y0 and y1 < lo + n_win
        R[i, y0 - lo] += 1.0 - fy
        R[i, y1 - lo] += fy
    return R


def _build_program():
    nc = bacc.Bacc("TRN2")

    def din(name, shape, dtype=F32):
        return nc.dram_tensor(name, list(shape), dtype, kind="ExternalInput")

    d_binfo = din("binfo", [B, NBC, 3])
    d_binfoT = din("binfoT", [3, B * NBC])
    d_lpre = din("lpre", [3, B * NBC])  # rows [bx, by, -0.5]; L3 = -2 * lpre
    d_e1w = din("e1w", [3, HID])
    d_e1b = din("e1b", [HID, 1])
    d_e2w = din("e2w", [HID, HID])
    d_e2b = din("e2b", [HID, 1])
    d_g1wf = din("g1wf", [HID, HID])
    d_g1b = din("g1b", [HID, 1])
    d_w4 = din("w4", [4, 128], BF16)
    d_g2bd = din("g2bd", [128, HID], BF16)
    d_g2b2 = din("g2b2", [128, 1])
    d_g3a = din("g3a", [128, 8], BF16)
    d_g3b_ = din("g3bm", [128, 8], BF16)
    d_g3b4 = din("g3b4", [4, 1])
    d_eye4 = din("eye4", [128, 16])
    d_cxd3 = din("cxd3", [3, MK])
    d_xcy = din("xcy", [2, MK], BF16)
    d_ryt = din("ryt", [RPC, OUT_ROWS])
    d_rx = din("rx", [HC, W])
    d_ds = din("ds", [1, 1])
    d_out = nc.dram_tensor("out", [B, OUT_ROWS, W], F32, kind="ExternalOutput")

    CH = [(0, 512), (512, 640)]  # PSUM-bank-sized free-dim chunks of MK

    with tile.TileContext(nc) as tc:
        with (
            tc.tile_pool(name="const", bufs=1) as cp,
            tc.tile_pool(name="persist", bufs=1) as pp,
        ):
            def cload(dram, shape, dtype=F32, name=None):
                t = cp.tile(shape, dtype, name=name or dram.name + "_sb")
                nc.sync.dma_start(out=t, in_=dram[:])
                return t

            sb_binfoT = cload(d_binfoT, [3, B * NBC])
            sb_lpre = cload(d_lpre, [3, B * NBC])
            sb_e1w = cload(d_e1w, [3, HID])
            sb_e1b = cload(d_e1b, [HID, 1])
            sb_e2w = cload(d_e2w, [HID, HID])
            sb_e2b = cload(d_e2b, [HID, 1])
            sb_g1wf = cload(d_g1wf, [HID, HID])
            sb_g1b = cload(d_g1b, [HID, 1])
            sb_w4 = cload(d_w4, [4, 128], BF16)
            sb_g2bd = cload(d_g2bd, [128, HID], BF16)
            sb_g2b2 = cload(d_g2b2, [128, 1])
            sb_g3a = cload(d_g3a, [128, 8], BF16)
            sb_g3b_ = cload(d_g3b_, [128, 8], BF16)
            sb_g3b4 = cload(d_g3b4, [4, 1])
            sb_eye4 = cload(d_eye4, [128, 16])
            sb_cxd3 = cload(d_cxd3, [3, MK])
            sb_ryt = cload(d_ryt, [RPC, OUT_ROWS])
            sb_rx = cload(d_rx, [HC, W])
            sb_binfo = cp.tile([NBC, B * 3], F32, name="binfo_sb")
            for b in range(B):
                nc.sync.dma_start(out=sb_binfo[:, 3 * b:3 * b + 3], in_=d_binfo[b])
            sb_s = cp.tile([128, 1], F32, name="s_sb")
            nc.sync.dma_start(
                out=sb_s,
                in_=bass.AP(tensor=d_ds, offset=0, ap=[[0, 128], [1, 1]]),
            )

            # persistent intermediates
            DW = [pp.tile([NBC, MK], F32, name=f"dw{b}") for b in range(B)]
            DBF = [pp.tile([NBC, MK], BF16, name=f"dbf{b}") for b in range(B)]
            RAW = [pp.tile([NBC, MK], F32, name=f"raw{b}") for b in range(B)]
            A_col = pp.tile([128, NPAIR], F32, name="a_col")
            # double-buffered XI (rhs of mm1): rows 0-1 = cx/cy (filled once),
            # rows 2-3 = per-group boundary-point distances
            XIT = [pp.tile([4, 32 * MK], BF16, name=f"xi{j}") for j in range(2)]
            for j in range(2):
                nc.sync.dma_start(
                    out=XIT[j].rearrange("r (q m) -> r q m", m=MK)[0:2],
                    in_=bass.AP(
                        tensor=d_xcy, offset=0, ap=[[MK, 2], [0, 32], [1, MK]]
                    ),
                )

            # ---------------- preamble: distances, then encoder ----------
            with (
                tc.tile_pool(name="pre_sb", bufs=2) as sp,
                tc.tile_pool(name="pre_ps", bufs=2, space="PSUM") as pq,
            ):
                # Dummy back-to-back matmuls that keep the PE HAM un-throttled
                # (2.4 GHz) through the DMA/ACT-heavy preamble; results unused.
                ps_warm = pq.tile([HID, HID], F32, name="ps_warm", tag="warm")

                def pe_keep_warm(n):
                    for _ in range(n):
                        nc.tensor.matmul(
                            ps_warm, lhsT=sb_g2bd, rhs=sb_g2bd, start=True, stop=True
                        )

                pe_keep_warm(64)

                # -|s| on all partitions
                s_abs = sp.tile([128, 1], F32, name="s_abs")
                nc.scalar.activation(s_abs, sb_s, AF.Abs)
                s_neg = sp.tile([128, 1], F32, name="s_neg")
                nc.vector.tensor_scalar_mul(s_neg, s_abs, -1.0)

                # L3 rows: [-2bx; -2by; ones]  over all 512 boundary points
                L3 = sp.tile([3, B * NBC], F32, name="L3")
                nc.vector.tensor_scalar_mul(L3, sb_lpre, -2.0)

                # per-partition bias bx^2 + by^2 + eps  (column per batch)
                bxy = sp.tile([NBC, B], F32, name="bxy")
                for b in range(B):
                    sq = sp.tile([NBC, 2], F32, name="sq")
                    nc.vector.tensor_mul(
                        sq, sb_binfo[:, 3 * b:3 * b + 2], sb_binfo[:, 3 * b:3 * b + 2]
                    )
                    nc.vector.tensor_reduce(
                        bxy[:, b:b + 1], sq, axis=mybir.AxisListType.X, op=ALU.add
                    )
                nc.vector.tensor_scalar_add(bxy, bxy, EPS)

                # dist2 -> dist -> dw (+bf16 cast of dist)
                dist32 = []
                ps_d = []
                for b in range(B):
                    ps = pq.tile([NBC, MK], F32, name="pps", tag="pps")
                    for lo, hi in CH:
                        nc.tensor.matmul(
                            ps[:, lo:hi],
                            lhsT=L3[:, NBC * b:NBC * (b + 1)],
                            rhs=sb_cxd3[:, lo:hi],
                            start=True,
                            stop=True,
                        )
                    ps_d.append(ps)
                for b in range(B):
                    dst = sp.tile([NBC, MK], F32, name=f"dist32_{b}", tag=f"d32_{b}")
                    nc.scalar.activation(
                        dst, ps_d[b], AF.Sqrt, bias=bxy[:, b:b + 1]
                    )
                    dist32.append(dst)
                for b in range(B):
                    nc.scalar.activation(
                        DW[b], dist32[b], AF.Exp, scale=s_neg[:, 0:1]
                    )
                for b in range(B):
                    nc.vector.tensor_copy(DBF[b], dist32[b])
                pe_keep_warm(48)

                # boundary encoder (fp32): bf = gelu(gelu(x@e1+b)@e2+b)
                ps1 = pq.tile([HID, B * NBC], F32, name="pps_e1", tag="pps")
                nc.tensor.matmul(ps1, lhsT=sb_e1w, rhs=sb_binfoT, start=True, stop=True)
                enc1 = sp.tile([HID, B * NBC], F32, name="enc1")
                nc.scalar.activation(enc1, ps1, AF.Gelu, bias=sb_e1b[:, 0:1])
                ps2 = pq.tile([HID, B * NBC], F32, name="pps_e2", tag="pps")
                nc.tensor.matmul(ps2, lhsT=sb_e2w, rhs=enc1, start=True, stop=True)
                bf = sp.tile([HID, B * NBC], F32, name="bf")
                nc.scalar.activation(bf, ps2, AF.Gelu, bias=sb_e2b[:, 0:1])
                ps3 = pq.tile([HID, B * NBC], F32, name="pps_a", tag="pps")
                nc.tensor.matmul(ps3, lhsT=sb_g1wf, rhs=bf, start=True, stop=True)
                A = sp.tile([HID, B * NBC], F32, name="A")
                nc.scalar.activation(A, ps3, AF.Identity, bias=sb_g1b[:, 0:1])

                # A_col [128, 256]: column p = concat(a[:, 2p], a[:, 2p+1])
                Av = A.rearrange("h (p two) -> h two p", two=2)
                nc.sync.dma_start(out=A_col[0:HID, :], in_=Av[:, 0, :])
                nc.sync.dma_start(out=A_col[HID:128, :], in_=Av[:, 1, :])
                pe_keep_warm(48)

            # ---------------- main loop ----------------------------------
            with (
                tc.tile_pool(name="h1p", bufs=2) as h1p,
                tc.tile_pool(name="h2wp", bufs=2) as h2wp,
                tc.tile_pool(name="stgp", bufs=4) as stgp,
                tc.tile_pool(name="ps_h1", bufs=2, space="PSUM") as psh1,
                tc.tile_pool(name="ps_h2", bufs=1, space="PSUM") as psh2,
                tc.tile_pool(name="ps_raw", bufs=1, space="PSUM") as psraw,
            ):
                for g in range(8):
                    b, half = g // 2, g % 2
                    xi = XIT[g % 2]
                    xiv = xi.rearrange("r (q m) -> r q m", m=MK)
                    dv = DBF[b][64 * half:64 * half + 64, :].rearrange(
                        "(q r) m -> q r m", r=2
                    )
                    nc.sync.dma_start(out=xiv[2:3], in_=dv[:, 0, :])
                    nc.sync.dma_start(out=xiv[3:4], in_=dv[:, 1, :])

                    ph2 = None
                    praw = None
                    for q in range(32):
                        pair = 32 * g + q
                        ph1 = psh1.tile([128, MK], F32, name="ph1", tag="ph1")
                        for lo, hi in CH:
                            nc.tensor.matmul(
                                ph1[:, lo:hi],
                                lhsT=sb_w4,
                                rhs=xi[:, MK * q + lo:MK * q + hi],
                                start=True,
                                stop=True,
                            )
                        h1 = h1p.tile([128, MK], BF16, name="h1", tag="h1")
                        nc.scalar.activation(
                            h1, ph1, AF.Gelu, bias=A_col[:, pair:pair + 1]
                        )
                        if q % 2 == 0:
                            ph2 = psh2.tile([128, MK], F32, name="ph2", tag="ph2")
                        p0 = 64 * (q % 2)
                        for lo, hi in CH:
                            nc.tensor.matmul(
                                ph2[p0:p0 + 64, lo:hi],
                                lhsT=sb_g2bd,
                                rhs=h1[:, lo:hi],
                                start=True,
                                stop=True,
                            )
                        if q % 2 == 1:
                            h2w = h2wp.tile([128, MK], BF16, name="h2w", tag="h2w")
                            nc.scalar.activation(
                                h2w, ph2, AF.Gelu, bias=sb_g2b2[:, 0:1]
                            )
                            if q % 4 == 1:
                                praw = psraw.tile([8, MK], F32, name="praw", tag="praw")
                            wsel = sb_g3a if q % 4 == 1 else sb_g3b_
                            for lo, hi in CH:
                                nc.tensor.matmul(
                                    praw[:, lo:hi],
                                    lhsT=wsel,
                                    rhs=h2w[:, lo:hi],
                                    start=(q % 4 == 1),
                                    stop=(q % 4 == 3),
                                    skip_group_check=True,
                                )
                            if q % 4 == 3:
                                stg = stgp.tile([8, MK], F32, name="stg", tag="stg")
                                nc.vector.tensor_copy(stg, praw)
                                r0 = 64 * half + 2 * (q - 3)
                                nc.sync.dma_start(
                                    out=RAW[b][r0:r0 + 8, :], in_=stg
                                )

            # ---------------- epilogue -----------------------------------
            with (
                tc.tile_pool(name="epi_sb", bufs=2) as ep,
                tc.tile_pool(name="epi_ps", bufs=1, space="PSUM") as eq,
            ):
                for b in range(B):
                    nc.vector.tensor_mul(RAW[b], RAW[b], DW[b])
                ps_u1 = eq.tile([B, MK], F32, name="ps_u1", tag="u1")
                ps_u2 = eq.tile([B, MK], F32, name="ps_u2", tag="u2")
                for b in range(B):
                    for lo, hi in CH:
                        nc.tensor.matmul(
                            ps_u1[:, lo:hi],
                            lhsT=sb_eye4[:, 4 * b:4 * b + 4],
                            rhs=RAW[b][:, lo:hi],
                            start=(b == 0),
                            stop=(b == B - 1),
                            skip_group_check=True,
                        )
                for b in range(B):
                    for lo, hi in CH:
                        nc.tensor.matmul(
                            ps_u2[:, lo:hi],
                            lhsT=sb_eye4[:, 4 * b:4 * b + 4],
                            rhs=DW[b][:, lo:hi],
                            start=(b == 0),
                            stop=(b == B - 1),
                            skip_group_check=True,
                        )
                u_sb = ep.tile([B, MK], F32, name="u_sb")
                nc.vector.tensor_scalar(
                    u_sb, ps_u2, sb_g3b4[:, 0:1], None, op0=ALU.mult
                )
                nc.vector.tensor_add(u_sb, u_sb, ps_u1)

                for b in range(B):
                    ub = ep.tile([RPC, WC], F32, name=f"ub{b}", tag="ub")
                    nc.sync.dma_start(out=ub, in_=u_sb[b:b + 1, :])
                    ps_c = eq.tile([WC, OUT_ROWS], F32, name="ps_c", tag="psc")
                    nc.tensor.matmul(ps_c, lhsT=ub, rhs=sb_ryt, start=True, stop=True)
                    c1t = ep.tile([WC, OUT_ROWS], F32, name="c1t", tag="c1t")
                    nc.vector.tensor_copy(c1t, ps_c)
                    ps_o = eq.tile([OUT_ROWS, W], F32, name="ps_o", tag="pso")
                    nc.tensor.matmul(ps_o, lhsT=c1t, rhs=sb_rx, start=True, stop=True)
                    o_sb = ep.tile([OUT_ROWS, W], F32, name=f"o{b}", tag="osb")
                    nc.vector.tensor_copy(o_sb, ps_o)
                    nc.sync.dma_start(out=d_out[b], in_=o_sb)

    nc.finalize()
    return nc


_CACHED = None


def _get_program():
    global _CACHED
    if _CACHED is None:
        _CACHED = _build_program()
    return _CACHED


def _make_in_maps(inputs):
    f32 = lambda x: np.ascontiguousarray(np.asarray(x), dtype=np.float32)
    b16 = lambda x: np.ascontiguousarray(
        np.asarray(x, dtype=np.float32).astype(ml_dtypes.bfloat16)
    )
    binfo = f32(inputs["boundary_info"])
    e1w, e1b = f32(inputs["e1w"]), f32(inputs["e1b"])
    e2w, e2b = f32(inputs["e2w"]), f32(inputs["e2b"])
    g1w, g1b = f32(inputs["g1w"]), f32(inputs["g1b"])
    g2w, g2b = f32(inputs["g2w"]), f32(inputs["g2b"])
    g3w, g3b = f32(inputs["g3w"]), f32(inputs["g3b"])
    ds = f32(inputs["distance_scale"])

    gxw, gyw, gdw = g1w[HID + 0], g1w[HID + 1], g1w[HID + 2]
    w4 = np.zeros((4, 128), np.float32)
    w4[0, :HID], w4[0, HID:] = gxw, gxw
    w4[1, :HID], w4[1, HID:] = gyw, gyw
    w4[2, :HID] = gdw
    w4[3, HID:] = gdw

    g2bd = np.zeros((128, HID), np.float32)
    g2bd[:HID, :32] = g2w
    g2bd[HID:, 32:] = g2w
    g2b2 = np.tile(g2b, 4)[:, None]

    g3a = np.zeros((128, 8), np.float32)
    g3bm = np.zeros((128, 8), np.float32)
    for j in range(4):
        g3a[32 * j:32 * j + 32, j] = g3w[:, 0]
        g3bm[32 * j:32 * j + 32, 4 + j] = g3w[:, 0]

    eye4 = np.zeros((128, 16), np.float32)
    for b in range(4):
        eye4[:, 4 * b + b] = 1.0

    gx = np.linspace(-1.0, 1.0, WC)
    gy = np.linspace(-1.0, 1.0, HC)
    rx = np.ascontiguousarray(
        _interp_matrix(range(W), WC, 0, WC, W).T.astype(np.float32)
    )  # [64, 256]

    binfoT = np.ascontiguousarray(binfo.reshape(B * NBC, 3).T)
    lpre = binfoT.copy()
    lpre[2, :] = -0.5
    shared = dict(
        binfo=binfo,
        binfoT=binfoT,
        lpre=lpre,
        e1w=e1w,
        e1b=np.ascontiguousarray(e1b[:, None]),
        e2w=e2w,
        e2b=np.ascontiguousarray(e2b[:, None]),
        g1wf=np.ascontiguousarray(g1w[:HID]),
        g1b=np.ascontiguousarray(g1b[:, None]),
        w4=b16(w4),
        g2bd=b16(g2bd),
        g2b2=f32(g2b2),
        g3a=b16(g3a),
        g3bm=b16(g3bm),
        g3b4=np.full((4, 1), g3b[0], np.float32),
        eye4=eye4,
        rx=rx,
        ds=ds.reshape(1, 1),
    )

    starts = _core_row_starts()
    in_maps = []
    for k in range(NCORES):
        sk = starts[k]
        rows = np.arange(sk, sk + RPC)
        cy = np.repeat(gy[rows], WC)
        cx = np.tile(gx, RPC)
        cxd3 = np.stack([cx, cy, cx * cx + cy * cy]).astype(np.float32)
        xcy = np.stack([cx, cy]).astype(np.float32)
        ryt = (
            _interp_matrix(range(OUT_ROWS * k, OUT_ROWS * (k + 1)), HC, sk, RPC, H)
            / NBC
        ).T.astype(np.float32)  # [10, 32]
        m = dict(shared)
        m.update(
            cxd3=np.ascontiguousarray(cxd3),
            xcy=b16(xcy),
            ryt=np.ascontiguousarray(ryt),
        )
        in_maps.append(m)
    return in_maps


def kernel(**inputs) -> np.ndarray:
    global LAST_RESULT
    assert int(inputs["H"]) == H and int(inputs["W"]) == W
    nc = _get_program()
    in_maps = _make_in_maps(inputs)
    res = run_bass_kernel_spmd(
        nc, in_maps, core_ids=list(range(NCORES)), trace=TRACE
    )
    LAST_RESULT = res
    shards = [r["out"] for r in res.results]  # each [B, 32, W]
    out = np.concatenate(shards, axis=1)      # [B, 256, W]
    return out[:, None, :, :].astype(np.float32)


# revision 25
# speedup vs baseline: 1.5055x; 1.0749x over previous
"""Trainium2 Bass kernel for nn_BoundaryGreenBranch.

Strategy (8 NeuronCores, full inputs in / full output out):
  - Shard the 64x64 coarse grid by rows: core k owns a 10-row window
    (640 coarse points, 2 rows of overlap so each core can run its own
    slice of the bilinear upsample -> zero cross-core communication) and
    produces output rows [32k, 32k+32) of the final [4,1,256,256].
  - Per core, all 512 (batch, boundary-point) pairs are processed with two
    boundary points stacked on the 128 partitions (2 x 64 hidden).  The
    green-kernel MLP runs entirely out of SBUF/PSUM (flash-style, nothing
    materialized in HBM):
      mm1   K=4  [cx; cy; d0; d1] x W4            -> h1_pre  [128, 640]
      gelu1 (+ per-pair bias a = bf@g1w_f + g1b, per-partition bias)
      mm2   K=128 blockdiag(g2w, g2w)             -> h2_pre  [64, 640]
      gelu2 (+ blockdiag bias)
      mm3   K=128 blockdiag4(g3w)                 -> raw     [8, 640] / 4 pairs
    Distances for all pairs are precomputed with one rank-3 matmul per batch
    plus Sqrt/Exp activations.  The weighted sum over boundary points is a
    single K=128 PE reduction per batch at the end, followed by the separable
    bilinear upsample done as two small matmuls per batch.
"""

import numpy as np
import ml_dtypes

import concourse.bass as bass
import concourse.mybir as mybir
import concourse.tile as tile
from concourse import bacc
from concourse.bass_utils import run_bass_kernel_spmd

B, NBC, HID = 4, 128, 64
H = W = 256
HC = WC = 64
CF = 4
NCORES = 8
RPC = 10                 # coarse rows per core (incl. upsample overlap)
MK = RPC * WC            # 640 coarse points per core
OUT_ROWS = H // NCORES   # 32 output rows per core
NPAIR = B * NBC // 2     # 256 pairs of boundary points
EPS = 1e-8

F32 = mybir.dt.float32
BF16 = mybir.dt.bfloat16
AF = mybir.ActivationFunctionType
ALU = mybir.AluOpType

LAST_RESULT = None       # BassKernelResults of the most recent run (for test.py)
TRACE = False            # set True by test.py to capture an NTFF profile


def _core_row_starts():
    starts = []
    for k in range(NCORES):
        s = (OUT_ROWS * k * (HC - 1)) // (H - 1)
        starts.append(min(s, HC - RPC))
    return starts


def _interp_matrix(out_idx, n_in, lo, n_win, n_out_total):
    out_idx = list(out_idx)
    R = np.zeros((len(out_idx), n_win), dtype=np.float64)
    for i, h in enumerate(out_idx):
        y = h * (n_in - 1) / (n_out_total - 1)
        y0 = int(np.floor(y))
        y1 = min(y0 + 1, n_in - 1)
        fy = y - y0
        assert lo <= y0 and y1 < lo + n_win
        R[i, y0 - lo] += 1.0 - fy
        R[i, y1 - lo] += fy
    return R


def _build_program():
    nc = bacc.Bacc("TRN2")

    def din(name, shape, dtype=F32):
        return nc.dram_tensor(name, list(shape), dtype, kind="ExternalInput")

    d_binfo = din("binfo", [B, NBC, 3])
    d_binfoT = din("binfoT", [3, B * NBC])
    d_binfoTe = din("binfoTe", [3, B * NBC])  # pair-permuted (even bn | odd bn)
    d_lpre = din("lpre", [3, B * NBC])  # rows [bx, by, -0.5]; L3 = -2 * lpre
    d_e1w = din("e1w", [3, HID])
    d_e1b = din("e1b", [HID, 1])
    d_e2w = din("e2w", [HID, HID])
    d_e2b = din("e2b", [HID, 1])
    d_g1wf = din("g1wf", [HID, HID])
    d_g1b = din("g1b", [HID, 1])
    d_w4 = din("w4", [4, 128], BF16)
    d_g2bd = din("g2bd", [128, HID], BF16)
    d_g2b2 = din("g2b2", [128, 1])
    d_g3a = din("g3a", [128, 8], BF16)
    d_g3b_ = din("g3bm", [128, 8], BF16)
    d_g3b4 = din("g3b4", [4, 1])
    d_eye4 = din("eye4", [128, 16], BF16)
    d_cxd3 = din("cxd3", [3, MK])
    d_xcyrep = din("xcyrep", [2, 32 * MK], BF16)
    d_ryt = din("ryt", [RPC, OUT_ROWS])
    d_rx = din("rx", [HC, W])
    d_ds = din("ds", [1, 1])
    d_out = nc.dram_tensor("out", [B, OUT_ROWS, W], F32, kind="ExternalOutput")

    CH = [(0, 512), (512, 640)]  # PSUM-bank-sized free-dim chunks of MK

    with tile.TileContext(nc) as tc:
        with (
            tc.tile_pool(name="const", bufs=1) as cp,
            tc.tile_pool(name="persist", bufs=1) as pp,
        ):
            def cload(dram, shape, dtype=F32, name=None):
                t = cp.tile(shape, dtype, name=name or dram.name + "_sb")
                nc.sync.dma_start(out=t, in_=dram[:])
                return t

            sb_binfoT = cload(d_binfoT, [3, B * NBC])
            sb_binfoTe = cload(d_binfoTe, [3, B * NBC])
            sb_lpre = cload(d_lpre, [3, B * NBC])
            sb_e1w = cload(d_e1w, [3, HID])
            sb_e1b = cload(d_e1b, [HID, 1])
            sb_e2w = cload(d_e2w, [HID, HID])
            sb_e2b = cload(d_e2b, [HID, 1])
            sb_g1wf = cload(d_g1wf, [HID, HID])
            sb_g1b = cload(d_g1b, [HID, 1])
            sb_w4 = cload(d_w4, [4, 128], BF16)
            sb_g2bd = cload(d_g2bd, [128, HID], BF16)
            sb_g2b2 = cload(d_g2b2, [128, 1])
            sb_g3a = cload(d_g3a, [128, 8], BF16)
            sb_g3b_ = cload(d_g3b_, [128, 8], BF16)
            sb_g3b4 = cload(d_g3b4, [4, 1])
            sb_eye4 = cload(d_eye4, [128, 16], BF16)
            sb_cxd3 = cload(d_cxd3, [3, MK])
            sb_ryt = cload(d_ryt, [RPC, OUT_ROWS])
            sb_rx = cload(d_rx, [HC, W])
            sb_binfo = cp.tile([NBC, B * 3], F32, name="binfo_sb")
            for b in range(B):
                nc.sync.dma_start(out=sb_binfo[:, 3 * b:3 * b + 3], in_=d_binfo[b])
            sb_s = cp.tile([128, 1], F32, name="s_sb")
            nc.sync.dma_start(
                out=sb_s,
                in_=bass.AP(tensor=d_ds, offset=0, ap=[[0, 128], [1, 1]]),
            )

            # persistent intermediates
            DW = [pp.tile([NBC, MK], BF16, name=f"dw{b}") for b in range(B)]
            DBF = [pp.tile([NBC, MK], BF16, name=f"dbf{b}") for b in range(B)]
            RAW = [pp.tile([NBC, MK], BF16, name=f"raw{b}") for b in range(B)]
            A_col = pp.tile([128, NPAIR], F32, name="a_col")
            # double-buffered XI (rhs of mm1): rows 0-1 = cx/cy (filled once),
            # rows 2-3 = per-group boundary-point distances
            XIT = [pp.tile([4, 32 * MK], BF16, name=f"xi{j}") for j in range(2)]
            for j in range(2):
                nc.sync.dma_start(out=XIT[j][0:2, :], in_=d_xcyrep[:])

            # ---------------- preamble: distances, then encoder ----------
            with (
                tc.tile_pool(name="pre_sb", bufs=2) as sp,
                tc.tile_pool(name="pre_ps", bufs=2, space="PSUM") as pq,
            ):
                # Dummy back-to-back matmuls that keep the PE HAM un-throttled
                # (2.4 GHz) through the DMA/ACT-heavy preamble; results unused.
                ps_warm = pq.tile([HID, HID], F32, name="ps_warm", tag="warm")

                def pe_keep_warm(n):
                    for _ in range(n):
                        nc.tensor.matmul(
                            ps_warm, lhsT=sb_g2bd, rhs=sb_g2bd, start=True, stop=True
                        )

                pe_keep_warm(64)

                # -|s| on all partitions
                s_abs = sp.tile([128, 1], F32, name="s_abs")
                nc.scalar.activation(s_abs, sb_s, AF.Abs)
                s_neg = sp.tile([128, 1], F32, name="s_neg")
                nc.vector.tensor_scalar_mul(s_neg, s_abs, -1.0)

                # L3 rows: [-2bx; -2by; ones]  over all 512 boundary points
                L3 = sp.tile([3, B * NBC], F32, name="L3")
                nc.vector.tensor_scalar_mul(L3, sb_lpre, -2.0)

                # per-partition bias bx^2 + by^2 + eps  (column per batch)
                bxy = sp.tile([NBC, B], F32, name="bxy")
                for b in range(B):
                    sq = sp.tile([NBC, 2], F32, name="sq")
                    nc.vector.tensor_mul(
                        sq, sb_binfo[:, 3 * b:3 * b + 2], sb_binfo[:, 3 * b:3 * b + 2]
                    )
                    nc.vector.tensor_reduce(
                        bxy[:, b:b + 1], sq, axis=mybir.AxisListType.X, op=ALU.add
                    )
                nc.vector.tensor_scalar_add(bxy, bxy, EPS)

                # dist2 -> dist -> dw (+bf16 cast of dist)
                dist32 = []
                ps_d = []
                for b in range(B):
                    ps = pq.tile([NBC, MK], F32, name="pps", tag="pps")
                    for lo, hi in CH:
                        nc.tensor.matmul(
                            ps[:, lo:hi],
                            lhsT=L3[:, NBC * b:NBC * (b + 1)],
                            rhs=sb_cxd3[:, lo:hi],
                            start=True,
                            stop=True,
                        )
                    ps_d.append(ps)
                for b in range(B):
                    dst = sp.tile([NBC, MK], F32, name=f"dist32_{b}", tag=f"d32_{b}")
                    nc.scalar.activation(
                        dst, ps_d[b], AF.Sqrt, bias=bxy[:, b:b + 1]
                    )
                    dist32.append(dst)
                for b in range(B):
                    nc.scalar.activation(
                        DW[b], dist32[b], AF.Exp, scale=s_neg[:, 0:1]
                    )
                for b in range(B):
                    nc.vector.tensor_copy(DBF[b], dist32[b])
                pe_keep_warm(48)

                # boundary encoder (fp32): bf = gelu(gelu(x@e1+b)@e2+b)
                ps1 = pq.tile([HID, B * NBC], F32, name="pps_e1", tag="pps")
                nc.tensor.matmul(ps1, lhsT=sb_e1w, rhs=sb_binfoTe, start=True, stop=True)
                enc1 = sp.tile([HID, B * NBC], F32, name="enc1")
                nc.scalar.activation(enc1, ps1, AF.Gelu, bias=sb_e1b[:, 0:1])
                ps2 = pq.tile([HID, B * NBC], F32, name="pps_e2", tag="pps")
                nc.tensor.matmul(ps2, lhsT=sb_e2w, rhs=enc1, start=True, stop=True)
                bf = sp.tile([HID, B * NBC], F32, name="bf")
                nc.scalar.activation(bf, ps2, AF.Gelu, bias=sb_e2b[:, 0:1])
                ps3 = pq.tile([HID, B * NBC], F32, name="pps_a", tag="pps")
                nc.tensor.matmul(ps3, lhsT=sb_g1wf, rhs=bf, start=True, stop=True)
                A = sp.tile([HID, B * NBC], F32, name="A")
                nc.scalar.activation(A, ps3, AF.Identity, bias=sb_g1b[:, 0:1])

                # A_col [128, 256]: column p = concat(a[:, 2p], a[:, 2p+1]);
                # encoder input was pair-permuted, so both halves are contiguous
                nc.sync.dma_start(out=A_col[0:HID, :], in_=A[:, 0:NPAIR])
                nc.sync.dma_start(out=A_col[HID:128, :], in_=A[:, NPAIR:2 * NPAIR])
                pe_keep_warm(96)

            # ---------------- main loop ----------------------------------
            with (
                tc.tile_pool(name="h1p", bufs=2) as h1p,
                tc.tile_pool(name="h2wp", bufs=2) as h2wp,
                tc.tile_pool(name="stgp", bufs=4) as stgp,
                tc.tile_pool(name="ps_h1", bufs=2, space="PSUM") as psh1,
                tc.tile_pool(name="ps_h2", bufs=1, space="PSUM") as psh2,
                tc.tile_pool(name="ps_raw", bufs=1, space="PSUM") as psraw,
            ):
                for g in range(8):
                    b, half = g // 2, g % 2
                    xi = XIT[g % 2]
                    xiv = xi.rearrange("r (q m) -> r q m", m=MK)
                    dv = DBF[b][64 * half:64 * half + 64, :].rearrange(
                        "(q r) m -> q r m", r=2
                    )
                    nc.sync.dma_start(out=xiv[2:3], in_=dv[:, 0, :])
                    nc.sync.dma_start(out=xiv[3:4], in_=dv[:, 1, :])

                    ph2 = None
                    praw = None
                    for q in range(32):
                        pair = 32 * g + q
                        ph1 = psh1.tile([128, MK], F32, name="ph1", tag="ph1")
                        for lo, hi in CH:
                            nc.tensor.matmul(
                                ph1[:, lo:hi],
                                lhsT=sb_w4,
                                rhs=xi[:, MK * q + lo:MK * q + hi],
                                start=True,
                                stop=True,
                            )
                        h1 = h1p.tile([128, MK], BF16, name="h1", tag="h1")
                        nc.scalar.activation(
                            h1, ph1, AF.Gelu, bias=A_col[:, pair:pair + 1]
                        )
                        if q % 2 == 0:
                            ph2 = psh2.tile([128, MK], F32, name="ph2", tag="ph2")
                        p0 = 64 * (q % 2)
                        for lo, hi in CH:
                            nc.tensor.matmul(
                                ph2[p0:p0 + 64, lo:hi],
                                lhsT=sb_g2bd,
                                rhs=h1[:, lo:hi],
                                start=True,
                                stop=True,
                            )
                        if q % 2 == 1:
                            h2w = h2wp.tile([128, MK], BF16, name="h2w", tag="h2w")
                            nc.scalar.activation(
                                h2w, ph2, AF.Gelu, bias=sb_g2b2[:, 0:1]
                            )
                            if q % 4 == 1:
                                praw = psraw.tile([8, MK], F32, name="praw", tag="praw")
                            wsel = sb_g3a if q % 4 == 1 else sb_g3b_
                            for lo, hi in CH:
                                nc.tensor.matmul(
                                    praw[:, lo:hi],
                                    lhsT=wsel,
                                    rhs=h2w[:, lo:hi],
                                    start=(q % 4 == 1),
                                    stop=(q % 4 == 3),
                                    skip_group_check=True,
                                )
                            if q % 4 == 3:
                                stg = stgp.tile([8, MK], BF16, name="stg", tag="stg")
                                nc.vector.tensor_copy(stg, praw)
                                r0 = 64 * half + 2 * (q - 3)
                                nc.sync.dma_start(
                                    out=RAW[b][r0:r0 + 8, :], in_=stg
                                )

            # ---------------- epilogue -----------------------------------
            with (
                tc.tile_pool(name="epi_sb", bufs=2) as ep,
                tc.tile_pool(name="epi_ps", bufs=1, space="PSUM") as eq,
            ):
                for b in range(B):
                    nc.vector.tensor_mul(RAW[b], RAW[b], DW[b])
                ps_u1 = eq.tile([B, MK], F32, name="ps_u1", tag="u1")
                ps_u2 = eq.tile([B, MK], F32, name="ps_u2", tag="u2")
                for b in range(B):
                    for lo, hi in CH:
                        nc.tensor.matmul(
                            ps_u1[:, lo:hi],
                            lhsT=sb_eye4[:, 4 * b:4 * b + 4],
                            rhs=RAW[b][:, lo:hi],
                            start=(b == 0),
                            stop=(b == B - 1),
                            skip_group_check=True,
                        )
                for b in range(B):
                    for lo, hi in CH:
                        nc.tensor.matmul(
                            ps_u2[:, lo:hi],
                            lhsT=sb_eye4[:, 4 * b:4 * b + 4],
                            rhs=DW[b][:, lo:hi],
                            start=(b == 0),
                            stop=(b == B - 1),
                            skip_group_check=True,
                        )
                u_sb = ep.tile([B, MK], F32, name="u_sb")
                nc.vector.tensor_scalar(
                    u_sb, ps_u2, sb_g3b4[:, 0:1], None, op0=ALU.mult
                )
                nc.vector.tensor_add(u_sb, u_sb, ps_u1)

                for b in range(B):
                    ub = ep.tile([RPC, WC], F32, name=f"ub{b}", tag="ub")
                    nc.sync.dma_start(out=ub, in_=u_sb[b:b + 1, :])
                    ps_c = eq.tile([WC, OUT_ROWS], F32, name="ps_c", tag="psc")
                    nc.tensor.matmul(ps_c, lhsT=ub, rhs=sb_ryt, start=True, stop=True)
                    c1t = ep.tile([WC, OUT_ROWS], F32, name="c1t", tag="c1t")
                    nc.vector.tensor_copy(c1t, ps_c)
                    ps_o = eq.tile([OUT_ROWS, W], F32, name="ps_o", tag="pso")
                    nc.tensor.matmul(ps_o, lhsT=c1t, rhs=sb_rx, start=True, stop=True)
                    o_sb = ep.tile([OUT_ROWS, W], F32, name=f"o{b}", tag="osb")
                    nc.vector.tensor_copy(o_sb, ps_o)
                    nc.sync.dma_start(out=d_out[b], in_=o_sb)

    nc.finalize()
    return nc


_CACHED = None


def _get_program():
    global _CACHED
    if _CACHED is None:
        _CACHED = _build_program()
    return _CACHED


def _make_in_maps(inputs):
    f32 = lambda x: np.ascontiguousarray(np.asarray(x), dtype=np.float32)
    b16 = lambda x: np.ascontiguousarray(
        np.asarray(x, dtype=np.float32).astype(ml_dtypes.bfloat16)
    )
    binfo = f32(inputs["boundary_info"])
    e1w, e1b = f32(inputs["e1w"]), f32(inputs["e1b"])
    e2w, e2b = f32(inputs["e2w"]), f32(inputs["e2b"])
    g1w, g1b = f32(inputs["g1w"]), f32(inputs["g1b"])
    g2w, g2b = f32(inputs["g2w"]), f32(inputs["g2b"])
    g3w, g3b = f32(inputs["g3w"]), f32(inputs["g3b"])
    ds = f32(inputs["distance_scale"])

    gxw, gyw, gdw = g1w[HID + 0], g1w[HID + 1], g1w[HID + 2]
    w4 = np.zeros((4, 128), np.float32)
    w4[0, :HID], w4[0, HID:] = gxw, gxw
    w4[1, :HID], w4[1, HID:] = gyw, gyw
    w4[2, :HID] = gdw
    w4[3, HID:] = gdw

    g2bd = np.zeros((128, HID), np.float32)
    g2bd[:HID, :32] = g2w
    g2bd[HID:, 32:] = g2w
    g2b2 = np.tile(g2b, 4)[:, None]

    g3a = np.zeros((128, 8), np.float32)
    g3bm = np.zeros((128, 8), np.float32)
    for j in range(4):
        g3a[32 * j:32 * j + 32, j] = g3w[:, 0]
        g3bm[32 * j:32 * j + 32, 4 + j] = g3w[:, 0]

    eye4 = np.zeros((128, 16), np.float32)
    for b in range(4):
        eye4[:, 4 * b + b] = 1.0

    gx = np.linspace(-1.0, 1.0, WC)
    gy = np.linspace(-1.0, 1.0, HC)
    rx = np.ascontiguousarray(
        _interp_matrix(range(W), WC, 0, WC, W).T.astype(np.float32)
    )  # [64, 256]

    binfoT = np.ascontiguousarray(binfo.reshape(B * NBC, 3).T)
    lpre = binfoT.copy()
    lpre[2, :] = -0.5
    perm = np.concatenate([np.arange(0, B * NBC, 2), np.arange(1, B * NBC, 2)])
    shared = dict(
        binfo=binfo,
        binfoT=binfoT,
        binfoTe=np.ascontiguousarray(binfoT[:, perm]),
        lpre=lpre,
        e1w=e1w,
        e1b=np.ascontiguousarray(e1b[:, None]),
        e2w=e2w,
        e2b=np.ascontiguousarray(e2b[:, None]),
        g1wf=np.ascontiguousarray(g1w[:HID]),
        g1b=np.ascontiguousarray(g1b[:, None]),
        w4=b16(w4),
        g2bd=b16(g2bd),
        g2b2=f32(g2b2),
        g3a=b16(g3a),
        g3bm=b16(g3bm),
        g3b4=np.full((4, 1), g3b[0], np.float32),
        eye4=b16(eye4),
        rx=rx,
        ds=ds.reshape(1, 1),
    )

    starts = _core_row_starts()
    in_maps = []
    for k in range(NCORES):
        sk = starts[k]
        rows = np.arange(sk, sk + RPC)
        cy = np.repeat(gy[rows], WC)
        cx = np.tile(gx, RPC)
        cxd3 = np.stack([cx, cy, cx * cx + cy * cy]).astype(np.float32)
        xcy = np.stack([cx, cy]).astype(np.float32)
        ryt = (
            _interp_matrix(range(OUT_ROWS * k, OUT_ROWS * (k + 1)), HC, sk, RPC, H)
            / NBC
        ).T.astype(np.float32)  # [10, 32]
        m = dict(shared)
        m.update(
            cxd3=np.ascontiguousarray(cxd3),
            xcyrep=b16(np.tile(xcy, (1, 32))),
            ryt=np.ascontiguousarray(ryt),
        )
        in_maps.append(m)
    return in_maps


def kernel(**inputs) -> np.ndarray:
    global LAST_RESULT
    assert int(inputs["H"]) == H and int(inputs["W"]) == W
    nc = _get_program()
    in_maps = _make_in_maps(inputs)
    res = run_bass_kernel_spmd(
        nc, in_maps, core_ids=list(range(NCORES)), trace=TRACE
    )
    LAST_RESULT = res
    shards = [r["out"] for r in res.results]  # each [B, 32, W]
    out = np.concatenate(shards, axis=1)      # [B, 256, W]
    return out[:, None, :, :].astype(np.float32)


# revision 26
# speedup vs baseline: 1.7100x; 1.1359x over previous
"""Trainium2 Bass kernel for nn_BoundaryGreenBranch.

Strategy (8 NeuronCores, full inputs in / full output out):
  - Shard the 64x64 coarse grid by rows: core k owns a 10-row window
    (640 coarse points, 2 rows of overlap so each core can run its own
    slice of the bilinear upsample -> zero cross-core communication) and
    produces output rows [32k, 32k+32) of the final [4,1,256,256].
  - Per core, all 512 (batch, boundary-point) pairs are processed with two
    boundary points stacked on the 128 partitions (2 x 64 hidden).  The
    green-kernel MLP runs entirely out of SBUF/PSUM (flash-style, nothing
    materialized in HBM):
      mm1   K=4  [cx; cy; d0; d1] x W4            -> h1_pre  [128, 640]
      gelu1 (+ per-pair bias a = bf@g1w_f + g1b, per-partition bias)
      mm2   K=128 blockdiag(g2w, g2w)             -> h2_pre  [64, 640]
      gelu2 (+ blockdiag bias)
      mm3   K=128 blockdiag4(g3w)                 -> raw     [8, 640] / 4 pairs
    Distances for all pairs are precomputed with one rank-3 matmul per batch
    plus Sqrt/Exp activations.  The weighted sum over boundary points is a
    single K=128 PE reduction per batch at the end, followed by the separable
    bilinear upsample done as two small matmuls per batch.
"""

import numpy as np
import ml_dtypes

import concourse.bass as bass
import concourse.mybir as mybir
import concourse.tile as tile
from concourse import bacc
from concourse.bass_utils import run_bass_kernel_spmd

B, NBC, HID = 4, 128, 64
H = W = 256
HC = WC = 64
CF = 4
NCORES = 8
RPC = 10                 # coarse rows per core (incl. upsample overlap)
MK = RPC * WC            # 640 coarse points per core
OUT_ROWS = H // NCORES   # 32 output rows per core
NPAIR = B * NBC // 2     # 256 pairs of boundary points
EPS = 1e-8

F32 = mybir.dt.float32
BF16 = mybir.dt.bfloat16
AF = mybir.ActivationFunctionType
ALU = mybir.AluOpType

LAST_RESULT = None       # BassKernelResults of the most recent run (for test.py)
TRACE = False            # set True by test.py to capture an NTFF profile


def _core_row_starts():
    starts = []
    for k in range(NCORES):
        s = (OUT_ROWS * k * (HC - 1)) // (H - 1)
        starts.append(min(s, HC - RPC))
    return starts


def _interp_matrix(out_idx, n_in, lo, n_win, n_out_total):
    out_idx = list(out_idx)
    R = np.zeros((len(out_idx), n_win), dtype=np.float64)
    for i, h in enumerate(out_idx):
        y = h * (n_in - 1) / (n_out_total - 1)
        y0 = int(np.floor(y))
        y1 = min(y0 + 1, n_in - 1)
        fy = y - y0
        assert lo <= y0 and y1 < lo + n_win
        R[i, y0 - lo] += 1.0 - fy
        R[i, y1 - lo] += fy
    return R


def _build_program():
    nc = bacc.Bacc("TRN2")

    def din(name, shape, dtype=F32):
        return nc.dram_tensor(name, list(shape), dtype, kind="ExternalInput")

    d_binfo = din("binfo", [B, NBC, 3])
    d_binfoT = din("binfoT", [3, B * NBC])
    d_binfoTe = din("binfoTe", [3, B * NBC])  # pair-permuted (even bn | odd bn)
    d_lpre = din("lpre", [3, B * NBC])  # rows [bx, by, -0.5]; L3 = -2 * lpre
    d_e1w = din("e1w", [3, HID])
    d_e1b = din("e1b", [HID, 1])
    d_e2w = din("e2w", [HID, HID])
    d_e2b = din("e2b", [HID, 1])
    d_g1wf = din("g1wf", [HID, HID])
    d_g1b = din("g1b", [HID, 1])
    d_w4 = din("w4", [4, 128], BF16)
    d_g2bd = din("g2bd", [128, HID], BF16)
    d_g2b2 = din("g2b2", [128, 1])
    d_g3a = din("g3a", [128, 8], BF16)
    d_g3b_ = din("g3bm", [128, 8], BF16)
    d_g3b4 = din("g3b4", [4, 1])
    d_eye4 = din("eye4", [128, 16], BF16)
    d_cxd3 = din("cxd3", [3, MK])
    d_xcyrep = din("xcyrep", [2, 32 * MK], BF16)
    d_ryt = din("ryt", [RPC, OUT_ROWS])
    d_rx = din("rx", [HC, W])
    d_ds = din("ds", [1, 1])
    d_out = nc.dram_tensor("out", [B, OUT_ROWS, W], F32, kind="ExternalOutput")

    CH = [(0, 512), (512, 640)]  # PSUM-bank-sized free-dim chunks of MK

    with tile.TileContext(nc) as tc:
        with (
            tc.tile_pool(name="const", bufs=1) as cp,
            tc.tile_pool(name="persist", bufs=1) as pp,
        ):
            def cload(dram, shape, dtype=F32, name=None):
                t = cp.tile(shape, dtype, name=name or dram.name + "_sb")
                nc.sync.dma_start(out=t, in_=dram[:])
                return t

            sb_binfoT = cload(d_binfoT, [3, B * NBC])
            sb_binfoTe = cload(d_binfoTe, [3, B * NBC])
            sb_lpre = cload(d_lpre, [3, B * NBC])
            sb_e1w = cload(d_e1w, [3, HID])
            sb_e1b = cload(d_e1b, [HID, 1])
            sb_e2w = cload(d_e2w, [HID, HID])
            sb_e2b = cload(d_e2b, [HID, 1])
            sb_g1wf = cload(d_g1wf, [HID, HID])
            sb_g1b = cload(d_g1b, [HID, 1])
            sb_w4 = cload(d_w4, [4, 128], BF16)
            sb_g2bd = cload(d_g2bd, [128, HID], BF16)
            sb_g2b2 = cload(d_g2b2, [128, 1])
            sb_g3a = cload(d_g3a, [128, 8], BF16)
            sb_g3b_ = cload(d_g3b_, [128, 8], BF16)
            sb_g3b4 = cload(d_g3b4, [4, 1])
            sb_eye4 = cload(d_eye4, [128, 16], BF16)
            sb_cxd3 = cload(d_cxd3, [3, MK])
            sb_ryt = cload(d_ryt, [RPC, OUT_ROWS])
            sb_rx = cload(d_rx, [HC, W])
            sb_binfo = cp.tile([NBC, B * 3], F32, name="binfo_sb")
            for b in range(B):
                nc.sync.dma_start(out=sb_binfo[:, 3 * b:3 * b + 3], in_=d_binfo[b])
            sb_s = cp.tile([128, 1], F32, name="s_sb")
            nc.sync.dma_start(
                out=sb_s,
                in_=bass.AP(tensor=d_ds, offset=0, ap=[[0, 128], [1, 1]]),
            )

            # persistent intermediates
            DW = [pp.tile([NBC, MK], BF16, name=f"dw{b}") for b in range(B)]
            DBF = [pp.tile([NBC, MK], BF16, name=f"dbf{b}") for b in range(B)]
            RAW = [pp.tile([NBC, MK], BF16, name=f"raw{b}") for b in range(B)]
            A_col = pp.tile([128, NPAIR], F32, name="a_col")
            # double-buffered XI (rhs of mm1): rows 0-1 = cx/cy (filled once),
            # rows 2-3 = per-group boundary-point distances
            XIT = [pp.tile([4, 32 * MK], BF16, name=f"xi{j}") for j in range(2)]
            for j in range(2):
                nc.sync.dma_start(out=XIT[j][0:2, :], in_=d_xcyrep[:])

            # ---------------- preamble: distances, then encoder ----------
            with (
                tc.tile_pool(name="pre_sb", bufs=2) as sp,
                tc.tile_pool(name="pre_ps", bufs=2, space="PSUM") as pq,
            ):
                # Dummy back-to-back matmuls that keep the PE HAM un-throttled
                # (2.4 GHz) through the DMA/ACT-heavy preamble; results unused.
                ps_warm = pq.tile([HID, HID], F32, name="ps_warm", tag="warm")

                def pe_keep_warm(n):
                    for _ in range(n):
                        nc.tensor.matmul(
                            ps_warm, lhsT=sb_g2bd, rhs=sb_g2bd, start=True, stop=True
                        )

                pe_keep_warm(64)

                # -|s| on all partitions
                s_abs = sp.tile([128, 1], F32, name="s_abs")
                nc.scalar.activation(s_abs, sb_s, AF.Abs)
                s_neg = sp.tile([128, 1], F32, name="s_neg")
                nc.vector.tensor_scalar_mul(s_neg, s_abs, -1.0)

                # L3 rows: [-2bx; -2by; ones]  over all 512 boundary points
                L3 = sp.tile([3, B * NBC], F32, name="L3")
                nc.vector.tensor_scalar_mul(L3, sb_lpre, -2.0)

                # per-partition bias bx^2 + by^2 + eps  (column per batch)
                bxy = sp.tile([NBC, B], F32, name="bxy")
                for b in range(B):
                    sq = sp.tile([NBC, 2], F32, name="sq")
                    nc.vector.tensor_mul(
                        sq, sb_binfo[:, 3 * b:3 * b + 2], sb_binfo[:, 3 * b:3 * b + 2]
                    )
                    nc.vector.tensor_reduce(
                        bxy[:, b:b + 1], sq, axis=mybir.AxisListType.X, op=ALU.add
                    )
                nc.vector.tensor_scalar_add(bxy, bxy, EPS)

                # dist2 -> dist -> dw (+bf16 cast of dist)
                dist32 = []
                ps_d = []
                for b in range(B):
                    ps = pq.tile([NBC, MK], F32, name="pps", tag="pps")
                    for lo, hi in CH:
                        nc.tensor.matmul(
                            ps[:, lo:hi],
                            lhsT=L3[:, NBC * b:NBC * (b + 1)],
                            rhs=sb_cxd3[:, lo:hi],
                            start=True,
                            stop=True,
                        )
                    ps_d.append(ps)
                for b in range(B):
                    dst = sp.tile([NBC, MK], F32, name=f"dist32_{b}", tag=f"d32_{b}")
                    nc.scalar.activation(
                        dst, ps_d[b], AF.Sqrt, bias=bxy[:, b:b + 1]
                    )
                    dist32.append(dst)
                for b in range(B):
                    nc.scalar.activation(
                        DW[b], dist32[b], AF.Exp, scale=s_neg[:, 0:1]
                    )
                for b in range(B):
                    nc.vector.tensor_copy(DBF[b], dist32[b])
                pe_keep_warm(48)

                # boundary encoder (fp32): bf = gelu(gelu(x@e1+b)@e2+b)
                ps1 = pq.tile([HID, B * NBC], F32, name="pps_e1", tag="pps")
                nc.tensor.matmul(ps1, lhsT=sb_e1w, rhs=sb_binfoTe, start=True, stop=True)
                enc1 = sp.tile([HID, B * NBC], F32, name="enc1")
                nc.scalar.activation(enc1, ps1, AF.Gelu, bias=sb_e1b[:, 0:1])
                ps2 = pq.tile([HID, B * NBC], F32, name="pps_e2", tag="pps")
                nc.tensor.matmul(ps2, lhsT=sb_e2w, rhs=enc1, start=True, stop=True)
                bf = sp.tile([HID, B * NBC], F32, name="bf")
                nc.scalar.activation(bf, ps2, AF.Gelu, bias=sb_e2b[:, 0:1])
                ps3 = pq.tile([HID, B * NBC], F32, name="pps_a", tag="pps")
                nc.tensor.matmul(ps3, lhsT=sb_g1wf, rhs=bf, start=True, stop=True)
                A = sp.tile([HID, B * NBC], F32, name="A")
                nc.scalar.activation(A, ps3, AF.Identity, bias=sb_g1b[:, 0:1])

                # A_col [128, 256]: column p = concat(a[:, 2p], a[:, 2p+1]);
                # encoder input was pair-permuted, so both halves are contiguous
                nc.sync.dma_start(out=A_col[0:HID, :], in_=A[:, 0:NPAIR])
                nc.sync.dma_start(out=A_col[HID:128, :], in_=A[:, NPAIR:2 * NPAIR])
                pe_keep_warm(96)

            # ---------------- main loop ----------------------------------
            with (
                tc.tile_pool(name="h1p", bufs=3) as h1p,
                tc.tile_pool(name="h2wp", bufs=3) as h2wp,
                tc.tile_pool(name="stgp", bufs=6) as stgp,
                tc.tile_pool(name="ps_h1", bufs=2, space="PSUM") as psh1,
                tc.tile_pool(name="ps_h2", bufs=1, space="PSUM") as psh2,
                tc.tile_pool(name="ps_raw", bufs=1, space="PSUM") as psraw,
            ):
                for g in range(8):
                    b, half = g // 2, g % 2
                    xi = XIT[g % 2]
                    xiv = xi.rearrange("r (q m) -> r q m", m=MK)
                    dv = DBF[b][64 * half:64 * half + 64, :].rearrange(
                        "(q r) m -> q r m", r=2
                    )
                    nc.sync.dma_start(out=xiv[2:3], in_=dv[:, 0, :])
                    nc.sync.dma_start(out=xiv[3:4], in_=dv[:, 1, :])

                    ph2 = None
                    praw = None
                    for q in range(32):
                        pair = 32 * g + q
                        ph1 = psh1.tile([128, MK], F32, name="ph1", tag="ph1")
                        for lo, hi in CH:
                            nc.tensor.matmul(
                                ph1[:, lo:hi],
                                lhsT=sb_w4,
                                rhs=xi[:, MK * q + lo:MK * q + hi],
                                start=True,
                                stop=True,
                            )
                        h1 = h1p.tile([128, MK], BF16, name="h1", tag="h1")
                        nc.scalar.activation(
                            h1, ph1, AF.Gelu, bias=A_col[:, pair:pair + 1]
                        )
                        if q % 2 == 0:
                            ph2 = psh2.tile([128, MK], F32, name="ph2", tag="ph2")
                        p0 = 64 * (q % 2)
                        for lo, hi in CH:
                            nc.tensor.matmul(
                                ph2[p0:p0 + 64, lo:hi],
                                lhsT=sb_g2bd,
                                rhs=h1[:, lo:hi],
                                start=True,
                                stop=True,
                            )
                        if q % 2 == 1:
                            h2w = h2wp.tile([128, MK], BF16, name="h2w", tag="h2w")
                            nc.scalar.activation(
                                h2w, ph2, AF.Gelu, bias=sb_g2b2[:, 0:1]
                            )
                            if q % 4 == 1:
                                praw = psraw.tile([8, MK], F32, name="praw", tag="praw")
                            wsel = sb_g3a if q % 4 == 1 else sb_g3b_
                            for lo, hi in CH:
                                nc.tensor.matmul(
                                    praw[:, lo:hi],
                                    lhsT=wsel,
                                    rhs=h2w[:, lo:hi],
                                    start=(q % 4 == 1),
                                    stop=(q % 4 == 3),
                                    skip_group_check=True,
                                )
                            if q % 4 == 3:
                                stg = stgp.tile([8, MK], BF16, name="stg", tag="stg")
                                nc.vector.tensor_copy(stg, praw)
                                r0 = 64 * half + 2 * (q - 3)
                                nc.sync.dma_start(
                                    out=RAW[b][r0:r0 + 8, :], in_=stg
                                )

            # ---------------- epilogue -----------------------------------
            with (
                tc.tile_pool(name="epi_sb", bufs=2) as ep,
                tc.tile_pool(name="epi_ps", bufs=1, space="PSUM") as eq,
            ):
                for b in range(B):
                    nc.vector.tensor_mul(RAW[b], RAW[b], DW[b])
                ps_u1 = eq.tile([B, MK], F32, name="ps_u1", tag="u1")
                ps_u2 = eq.tile([B, MK], F32, name="ps_u2", tag="u2")
                for b in range(B):
                    for lo, hi in CH:
                        nc.tensor.matmul(
                            ps_u1[:, lo:hi],
                            lhsT=sb_eye4[:, 4 * b:4 * b + 4],
                            rhs=RAW[b][:, lo:hi],
                            start=(b == 0),
                            stop=(b == B - 1),
                            skip_group_check=True,
                        )
                for b in range(B):
                    for lo, hi in CH:
                        nc.tensor.matmul(
                            ps_u2[:, lo:hi],
                            lhsT=sb_eye4[:, 4 * b:4 * b + 4],
                            rhs=DW[b][:, lo:hi],
                            start=(b == 0),
                            stop=(b == B - 1),
                            skip_group_check=True,
                        )
                u_sb = ep.tile([B, MK], F32, name="u_sb")
                nc.vector.tensor_scalar(
                    u_sb, ps_u2, sb_g3b4[:, 0:1], None, op0=ALU.mult
                )
                nc.vector.tensor_add(u_sb, u_sb, ps_u1)

                for b in range(B):
                    ub = ep.tile([RPC, WC], F32, name=f"ub{b}", tag="ub")
                    nc.sync.dma_start(out=ub, in_=u_sb[b:b + 1, :])
                    ps_c = eq.tile([WC, OUT_ROWS], F32, name="ps_c", tag="psc")
                    nc.tensor.matmul(ps_c, lhsT=ub, rhs=sb_ryt, start=True, stop=True)
                    c1t = ep.tile([WC, OUT_ROWS], F32, name="c1t", tag="c1t")
                    nc.vector.tensor_copy(c1t, ps_c)
                    ps_o = eq.tile([OUT_ROWS, W], F32, name="ps_o", tag="pso")
                    nc.tensor.matmul(ps_o, lhsT=c1t, rhs=sb_rx, start=True, stop=True)
                    o_sb = ep.tile([OUT_ROWS, W], F32, name=f"o{b}", tag="osb")
                    nc.vector.tensor_copy(o_sb, ps_o)
                    nc.sync.dma_start(out=d_out[b], in_=o_sb)

    nc.finalize()
    return nc


_CACHED = None


def _get_program():
    global _CACHED
    if _CACHED is None:
        _CACHED = _build_program()
    return _CACHED


def _make_in_maps(inputs):
    f32 = lambda x: np.ascontiguousarray(np.asarray(x), dtype=np.float32)
    b16 = lambda x: np.ascontiguousarray(
        np.asarray(x, dtype=np.float32).astype(ml_dtypes.bfloat16)
    )
    binfo = f32(inputs["boundary_info"])
    e1w, e1b = f32(inputs["e1w"]), f32(inputs["e1b"])
    e2w, e2b = f32(inputs["e2w"]), f32(inputs["e2b"])
    g1w, g1b = f32(inputs["g1w"]), f32(inputs["g1b"])
    g2w, g2b = f32(inputs["g2w"]), f32(inputs["g2b"])
    g3w, g3b = f32(inputs["g3w"]), f32(inputs["g3b"])
    ds = f32(inputs["distance_scale"])

    gxw, gyw, gdw = g1w[HID + 0], g1w[HID + 1], g1w[HID + 2]
    w4 = np.zeros((4, 128), np.float32)
    w4[0, :HID], w4[0, HID:] = gxw, gxw
    w4[1, :HID], w4[1, HID:] = gyw, gyw
    w4[2, :HID] = gdw
    w4[3, HID:] = gdw

    g2bd = np.zeros((128, HID), np.float32)
    g2bd[:HID, :32] = g2w
    g2bd[HID:, 32:] = g2w
    g2b2 = np.tile(g2b, 4)[:, None]

    g3a = np.zeros((128, 8), np.float32)
    g3bm = np.zeros((128, 8), np.float32)
    for j in range(4):
        g3a[32 * j:32 * j + 32, j] = g3w[:, 0]
        g3bm[32 * j:32 * j + 32, 4 + j] = g3w[:, 0]

    eye4 = np.zeros((128, 16), np.float32)
    for b in range(4):
        eye4[:, 4 * b + b] = 1.0

    gx = np.linspace(-1.0, 1.0, WC)
    gy = np.linspace(-1.0, 1.0, HC)
    rx = np.ascontiguousarray(
        _interp_matrix(range(W), WC, 0, WC, W).T.astype(np.float32)
    )  # [64, 256]

    binfoT = np.ascontiguousarray(binfo.reshape(B * NBC, 3).T)
    lpre = binfoT.copy()
    lpre[2, :] = -0.5
    perm = np.concatenate([np.arange(0, B * NBC, 2), np.arange(1, B * NBC, 2)])
    shared = dict(
        binfo=binfo,
        binfoT=binfoT,
        binfoTe=np.ascontiguousarray(binfoT[:, perm]),
        lpre=lpre,
        e1w=e1w,
        e1b=np.ascontiguousarray(e1b[:, None]),
        e2w=e2w,
        e2b=np.ascontiguousarray(e2b[:, None]),
        g1wf=np.ascontiguousarray(g1w[:HID]),
        g1b=np.ascontiguousarray(g1b[:, None]),
        w4=b16(w4),
        g2bd=b16(g2bd),
        g2b2=f32(g2b2),
        g3a=b16(g3a),
        g3bm=b16(g3bm),
        g3b4=np.full((4, 1), g3b[0], np.float32),
        eye4=b16(eye4),
        rx=rx,
        ds=ds.reshape(1, 1),
    )

    starts = _core_row_starts()
    in_maps = []
    for k in range(NCORES):
        sk = starts[k]
        rows = np.arange(sk, sk + RPC)
        cy = np.repeat(gy[rows], WC)
        cx = np.tile(gx, RPC)
        cxd3 = np.stack([cx, cy, cx * cx + cy * cy]).astype(np.float32)
        xcy = np.stack([cx, cy]).astype(np.float32)
        ryt = (
            _interp_matrix(range(OUT_ROWS * k, OUT_ROWS * (k + 1)), HC, sk, RPC, H)
            / NBC
        ).T.astype(np.float32)  # [10, 32]
        m = dict(shared)
        m.update(
            cxd3=np.ascontiguousarray(cxd3),
            xcyrep=b16(np.tile(xcy, (1, 32))),
            ryt=np.ascontiguousarray(ryt),
        )
        in_maps.append(m)
    return in_maps


def kernel(**inputs) -> np.ndarray:
    global LAST_RESULT
    assert int(inputs["H"]) == H and int(inputs["W"]) == W
    nc = _get_program()
    in_maps = _make_in_maps(inputs)
    res = run_bass_kernel_spmd(
        nc, in_maps, core_ids=list(range(NCORES)), trace=TRACE
    )
    LAST_RESULT = res
    shards = [r["out"] for r in res.results]  # each [B, 32, W]
    out = np.concatenate(shards, axis=1)      # [B, 256, W]
    return out[:, None, :, :].astype(np.float32)
